# revision 15
# baseline (speedup 1.0000x reference)
"""CTC greedy search Trainium2 kernel (8-core data parallel over batch).

Problem: logits (T=2048, N=32, V=1024) f32, in_lens (N,) int.
Returns (max_total f32 (N,), paths i32 (T, N), out_lens i32 (N,)).

Sharding: batch N split 4-per-core across 8 cores; everything else local.

Per-core algorithm:
  phase 1 (per [128, 1024] tile; rows are (n, t) pairs with t = 16*p + tc):
    - DMA tile in
    - ACT: exp(x) with accum -> sum_exp per row (raw exp is safe: |x| <= ~6)
    - DVE: max8 -> row max (top-8, we use [0]); max_index -> argmax (first occurrence)
  phase 1.5 (per n): maxlogp = max - ln(sum_exp); reshape [128,16] -> [1,2048]
    via cross-partition DMA so each n's t-sequence is one partition row.
  phase 2 (rows [4, 2048]): masks, dedup, cumsum (tensor_tensor_scan),
    compaction via two gpsimd local_scatter calls (dst halves of 1024, using
    the negative-index-is-ignored rule), merge with original argmax for the
    "undefined" tail, DMA out.
"""

import sys

if "/opt/trn_rl_repo" not in sys.path:
    sys.path.insert(0, "/opt/trn_rl_repo")

import numpy as np

T = 2048
N = 32
V = 1024
NCORES = 8
NLOC = N // NCORES  # 4
NT = 16             # t-chunks per n; t = 16*p + tc
BLANK = V - 1       # 1023

_BUILT = {}


def build_nc():
    import concourse.bass as bass
    import concourse.mybir as mybir
    from concourse.bacc import Bacc
    from concourse.tile import TileContext

    f32 = mybir.dt.float32
    i32 = mybir.dt.int32
    u32 = mybir.dt.uint32
    i16 = mybir.dt.int16
    Alu = mybir.AluOpType
    AFT = mybir.ActivationFunctionType

    nc = Bacc()
    lg = nc.declare_dram_parameter("logits", [T, NLOC, V], f32, isOutput=False)
    ll = nc.declare_dram_parameter("lens_f32", [NLOC, 1], f32, isOutput=False)
    io = nc.declare_dram_parameter("iota_f32", [16, T], f32, isOutput=False)
    paths_o = nc.declare_dram_parameter("paths", [NLOC, T], i32, isOutput=True)
    mt_o = nc.declare_dram_parameter("max_total", [NLOC, 1], f32, isOutput=True)
    ol_o = nc.declare_dram_parameter("out_lens", [NLOC, 1], i32, isOutput=True)

    # logits (t, n, v) viewed as [p, tc, n, v] with t = 16*p + tc
    lg_v = lg.ap().rearrange("(p s) n v -> p s n v", s=NT)

    with TileContext(nc) as tc_ctx:
        tc = tc_ctx
        with (
            tc.tile_pool(name="xp", bufs=8) as xpool,
            tc.tile_pool(name="ep", bufs=3) as epool,
            tc.tile_pool(name="res", bufs=1) as rpool,
            tc.tile_pool(name="p2", bufs=1) as p2pool,
        ):
            # persistent result tiles; column k = n*NT + tc
            NK = NLOC * NT
            CH = 256          # gather chunk (elements); 1 KB
            NCH = V // CH     # 4 chunks per row
            mx8 = rpool.tile([128, NK * 8], f32, tag="mx8", name="mx8")
            colmax = rpool.tile([128, NK * 8], f32, tag="colmax", name="colmax")
            c48 = rpool.tile([128, NK * 8], u32, tag="c48", name="c48")
            w8 = rpool.tile([128, NK * 8], u32, tag="w8", name="w8")
            se = rpool.tile([128, NK], f32, tag="se", name="se")
            # cols 4..7 of every colmax slot stay at -FLT_MAX so max8/max_index
            # over the 8-wide slot only see the NCH=4 real chunk maxes
            nc.gpsimd.memset(colmax[:], -3.0e38)

            # base_all[p, (n, tc)] = 256*p + 16*tc + 4*n: the 1KB-chunk id of
            # row (t=16p+tc, n) is base + c4 (row id t*4+n, 4 chunks per row)
            base_all = rpool.tile([128, NLOC, NT], i32, tag="base_all", name="base_all")
            nc.gpsimd.iota(
                base_all[:], pattern=[[4, NLOC], [16, NT]], base=0,
                channel_multiplier=CH,
            )

            # ---- phase 1 ----
            for n in range(NLOC):
                for tch in range(NT):
                    xt = xpool.tile([128, V], f32, tag="x")
                    nc.sync.dma_start(out=xt[:], in_=lg_v[:, tch, n, :])
                    et = epool.tile([128, V], f32, tag="e")
                    k = n * NT + tch
                    nc.scalar.activation(
                        et[:], xt[:], AFT.Exp,
                        accum_out=se[:, k : k + 1],
                    )
                    # hierarchical x-domain max/argmax: 4 chunk-maxes, then
                    # top-8 of the slot, then the index of the max chunk
                    xv = xt[:].rearrange("p (c e) -> p c e", c=NCH)
                    nc.vector.reduce_max(
                        colmax[:, k * 8 : k * 8 + NCH], xv, axis=mybir.AxisListType.X
                    )
                    mxv = mx8[:, k * 8 : (k + 1) * 8]
                    cmv = colmax[:, k * 8 : (k + 1) * 8]
                    nc.vector.max(mxv, cmv)
                    nc.vector.max_index(c48[:, k * 8 : (k + 1) * 8], mxv, cmv)

            # ---- phase 1b: gather the winning 1KB chunk of every row and
            # find the within-chunk argmax ----
            c4s = c48[:].rearrange("p (s e) -> p s e", e=8)[:, :, 0]
            g16 = rpool.tile([128, NK], i16, tag="g16", name="g16")
            nc.vector.scalar_tensor_tensor(
                g16[:], c4s, 0, base_all[:].rearrange("p a b -> p (a b)"),
                Alu.add, Alu.add,
            )
            # rewrap [128p, 64k] -> [16q, 64k, 8s] with gw[q,k,s] = g16[16s+q, k]
            # (dma_gather wants idx j of a tile at [j % 16, j // 16])
            with tc.tile_pool(name="dramp", bufs=1, space="DRAM") as dpool:
                gd = dpool.tile([128, NK], i16, tag="gd", name="gd")
                nc.sync.dma_start(out=gd[:], in_=g16[:])
                # idxs must be replicated into each Q7 core's 16 partitions
                gw = rpool.tile([128, NK * 8], i16, tag="gw", name="gw")
                gd_w = bass.AP(
                    gd[:].tensor, 0,
                    [[NK, 16], [1, NK], [16 * NK, 8]],
                )
                for r in range(8):
                    nc.sync.dma_start(
                        out=gw[16 * r : 16 * (r + 1), :].rearrange(
                            "q (k s) -> q k s", s=8
                        ),
                        in_=gd_w,
                    )
            lg_flat = lg.ap().rearrange("t n (c e) -> (t n c) e", e=CH)
            with tc.tile_pool(name="gp", bufs=8) as gpool:
                for k in range(NK):
                    gt = gpool.tile([128, CH], f32, tag="g")
                    nc.gpsimd.dma_gather(
                        gt[:].rearrange("p (a e) -> p a e", a=1),
                        lg_flat,
                        gw[:, k * 8 : (k + 1) * 8],
                        num_idxs=128,
                        num_idxs_reg=128,
                        elem_size=CH,
                    )
                    nc.vector.max_index(
                        w8[:, k * 8 : (k + 1) * 8], mx8[:, k * 8 : (k + 1) * 8], gt[:]
                    )

            # ---- phase 1.5: batched epilogue + reshape to [n, t] rows ----
            ami_nt = p2pool.tile([16, T], f32, tag="ami_nt")
            logp_nt = p2pool.tile([16, T], f32, tag="logp_nt")
            lnse = rpool.tile([128, NK], f32, tag="lnse", name="lnse")
            nc.scalar.activation(lnse[:], se[:], AFT.Ln)
            mxs = mx8[:].rearrange("p (s e) -> p s e", e=8)[:, :, 0]
            logp = rpool.tile([128, NK], f32, tag="logp", name="logp")
            # logp = maxlogp = max(x) - ln(sum e^x)
            nc.vector.scalar_tensor_tensor(
                logp[:], mxs, 0.0, lnse[:], Alu.add, Alu.subtract
            )
            # argmax = c4*256 + w, emitted directly as f32
            amif = rpool.tile([128, NK], f32, tag="amif", name="amif")
            ws = w8[:].rearrange("p (s e) -> p s e", e=8)[:, :, 0]
            nc.vector.scalar_tensor_tensor(
                amif[:], c4s, CH, ws, Alu.mult, Alu.add
            )
            for n in range(NLOC):
                # [128, 16] (p-major, tc-minor) -> one row of 2048 (t = 16p+tc)
                nc.sync.dma_start(
                    out=ami_nt[n : n + 1, :], in_=amif[:, n * NT : (n + 1) * NT]
                )
                nc.sync.dma_start(
                    out=logp_nt[n : n + 1, :], in_=logp[:, n * NT : (n + 1) * NT]
                )

            # ---- phase 2 ----
            lens_sb = p2pool.tile([16, 1], f32, tag="lens_sb")
            nc.sync.dma_start(out=lens_sb[0:NLOC, :], in_=ll.ap())

            it = p2pool.tile([16, T], f32, tag="iota", name="it")
            nc.sync.dma_start(out=it[:], in_=io.ap())

            def t4(tag, dt=f32, w=T):
                return p2pool.tile([16, w], dt, tag=tag, name=tag)

            lm = t4("lm")
            nc.vector.tensor_scalar(lm[:NLOC], it[:NLOC], lens_sb[:NLOC, :], None, Alu.is_lt)

            nb = t4("nb")
            nc.vector.scalar_tensor_tensor(
                nb[:NLOC], ami_nt[:NLOC], float(BLANK), lm[:NLOC], Alu.not_equal, Alu.mult
            )

            neq = t4("neq")
            nc.vector.memset(neq[:NLOC, 0:1], 1.0)
            nc.vector.scalar_tensor_tensor(
                neq[:NLOC, 1:], ami_nt[:NLOC, 1:], 0.0, ami_nt[:NLOC, : T - 1],
                Alu.add, Alu.not_equal,
            )

            keep = t4("keep")
            olf = p2pool.tile([16, 1], f32, tag="olf")
            nc.vector.scalar_tensor_tensor(
                keep[:NLOC], nb[:NLOC], 0.0, neq[:NLOC], Alu.add, Alu.mult,
                accum_out=olf[:NLOC, :],
            )

            scrap = t4("scrap")
            mts = p2pool.tile([16, 1], f32, tag="mts")
            nc.vector.scalar_tensor_tensor(
                scrap[:NLOC], logp_nt[:NLOC], 0.0, lm[:NLOC], Alu.add, Alu.mult,
                accum_out=mts[:NLOC, :],
            )

            cum = t4("cum")
            nc.vector.tensor_tensor_scan(
                cum[:NLOC], keep[:NLOC], keep[:NLOC], 0.0, Alu.add, Alu.bypass
            )

            # safe_pos + 1 = keep * cum  (0 where dropped, pos+1 where kept)
            spp1 = t4("spp1")
            nc.vector.scalar_tensor_tensor(
                spp1[:NLOC], keep[:NLOC], 0.0, cum[:NLOC], Alu.add, Alu.mult
            )

            idx_hi = t4("idx_hi", i16)
            nc.gpsimd.memset(idx_hi[:, :], -1)
            nc.vector.tensor_scalar(idx_hi[:NLOC], spp1[:NLOC], 1025.0, None, Alu.subtract)

            mhi = t4("mhi")
            nc.vector.tensor_scalar(mhi[:NLOC], spp1[:NLOC], 1025.0, None, Alu.is_ge)
            tlo = t4("tlo")
            nc.vector.scalar_tensor_tensor(
                tlo[:NLOC], mhi[:NLOC], -2048.0, spp1[:NLOC], Alu.mult, Alu.add
            )
            idx_lo = t4("idx_lo", i16)
            nc.gpsimd.memset(idx_lo[:, :], -1)
            nc.vector.tensor_scalar(idx_lo[:NLOC], tlo[:NLOC], 1.0, None, Alu.subtract)

            dat16 = t4("dat16", i16)
            nc.gpsimd.memset(dat16[:, :], 0)
            nc.vector.tensor_copy(dat16[:NLOC], ami_nt[:NLOC])

            cmp16 = t4("cmp16", i16)
            nc.gpsimd.local_scatter(
                cmp16[:, : T // 2], dat16[:], idx_lo[:],
                channels=16, num_elems=T // 2, num_idxs=T,
            )
            nc.gpsimd.local_scatter(
                cmp16[:, T // 2 :], dat16[:], idx_hi[:],
                channels=16, num_elems=T // 2, num_idxs=T,
            )

            cmpf = t4("cmpf")
            nc.vector.tensor_copy(cmpf[:NLOC], cmp16[:NLOC])

            msel = t4("msel", i32)
            nc.vector.tensor_scalar(msel[:NLOC], it[:NLOC], olf[:NLOC, :], None, Alu.is_lt)

            pf = t4("pf")
            nc.vector.select(pf[:NLOC], msel[:NLOC], cmpf[:NLOC], ami_nt[:NLOC])

            pi = t4("pi", i32)
            nc.vector.tensor_copy(pi[:NLOC], pf[:NLOC])
            oli = p2pool.tile([16, 1], i32, tag="oli")
            nc.vector.tensor_copy(oli[:NLOC], olf[:NLOC, :])

            nc.sync.dma_start(out=paths_o.ap(), in_=pi[:NLOC, :])
            nc.sync.dma_start(out=mt_o.ap(), in_=mts[:NLOC, :])
            nc.sync.dma_start(out=ol_o.ap(), in_=oli[:NLOC, :])

    return nc


def _get_nc():
    if "nc" not in _BUILT:
        nc = build_nc()
        nc.finalize()
        _BUILT["nc"] = nc
    return _BUILT["nc"]


_IOTA = np.broadcast_to(np.arange(T, dtype=np.float32), (16, T)).copy()


def make_in_maps(logits, in_lens):
    logits = np.ascontiguousarray(np.asarray(logits, dtype=np.float32))
    lens = np.asarray(in_lens).astype(np.float32).reshape(N)
    in_maps = []
    for c in range(NCORES):
        sl = slice(NLOC * c, NLOC * (c + 1))
        in_maps.append(
            {
                "logits": np.ascontiguousarray(logits[:, sl, :]),
                "lens_f32": np.ascontiguousarray(lens[sl].reshape(NLOC, 1)),
                "iota_f32": _IOTA,
            }
        )
    return in_maps


def kernel(logits, in_lens):
    from concourse.bass_utils import run_bass_kernel_spmd

    nc = _get_nc()
    in_maps = make_in_maps(logits, in_lens)
    res = run_bass_kernel_spmd(nc, in_maps, core_ids=list(range(NCORES))).results

    mt = np.concatenate([np.asarray(r["max_total"]).reshape(NLOC) for r in res])
    ol = np.concatenate([np.asarray(r["out_lens"]).reshape(NLOC) for r in res])
    paths = np.concatenate(
        [np.asarray(r["paths"]).reshape(NLOC, T) for r in res], axis=0
    )
    return (
        mt.astype(np.float32),
        np.ascontiguousarray(paths.T).astype(np.int32),
        ol.astype(np.int32),
    )


# revision 19
# speedup vs baseline: 1.2773x; 1.2773x over previous
"""CTC greedy search Trainium2 kernel (8-core data parallel over batch).

Problem: logits (T=2048, N=32, V=1024) f32, in_lens (N,) int.
Returns (max_total f32 (N,), paths i32 (T, N), out_lens i32 (N,)).

Sharding: batch N split 4-per-core across 8 cores; everything else local.

Per-core algorithm:
  phase 1 (per [128, 1024] tile; rows are (n, t) pairs with t = 16*p + tc):
    - DMA tile in
    - ACT: exp(x) with accum -> sum_exp per row (raw exp is safe: |x| <= ~6)
    - DVE: max8 -> row max (top-8, we use [0]); max_index -> argmax (first occurrence)
  phase 1.5 (per n): maxlogp = max - ln(sum_exp); reshape [128,16] -> [1,2048]
    via cross-partition DMA so each n's t-sequence is one partition row.
  phase 2 (rows [4, 2048]): masks, dedup, cumsum (tensor_tensor_scan),
    compaction via two gpsimd local_scatter calls (dst halves of 1024, using
    the negative-index-is-ignored rule), merge with original argmax for the
    "undefined" tail, DMA out.
"""

import sys

if "/opt/trn_rl_repo" not in sys.path:
    sys.path.insert(0, "/opt/trn_rl_repo")

import numpy as np

T = 2048
N = 32
V = 1024
NCORES = 8
NLOC = N // NCORES  # 4
NT = 16             # t-chunks per n; t = 16*p + tc
BLANK = V - 1       # 1023

_BUILT = {}


def build_nc():
    import concourse.bass as bass
    import concourse.mybir as mybir
    from concourse.bacc import Bacc
    from concourse.tile import TileContext

    f32 = mybir.dt.float32
    i32 = mybir.dt.int32
    u32 = mybir.dt.uint32
    i16 = mybir.dt.int16
    Alu = mybir.AluOpType
    AFT = mybir.ActivationFunctionType

    nc = Bacc()
    lg = nc.declare_dram_parameter("logits", [T, NLOC, V], f32, isOutput=False)
    ll = nc.declare_dram_parameter("lens_f32", [NLOC, 1], f32, isOutput=False)
    io = nc.declare_dram_parameter("iota_f32", [16, T], f32, isOutput=False)
    paths_o = nc.declare_dram_parameter("paths", [NLOC, T], i32, isOutput=True)
    mt_o = nc.declare_dram_parameter("max_total", [NLOC, 1], f32, isOutput=True)
    ol_o = nc.declare_dram_parameter("out_lens", [NLOC, 1], i32, isOutput=True)

    # logits (t, n, v) viewed as [p, tc, n, v] with t = 16*p + tc
    lg_v = lg.ap().rearrange("(p s) n v -> p s n v", s=NT)

    with TileContext(nc) as tc_ctx:
        tc = tc_ctx
        with (
            tc.tile_pool(name="xp", bufs=8) as xpool,
            tc.tile_pool(name="ep", bufs=3) as epool,
            tc.tile_pool(name="res", bufs=1) as rpool,
            tc.tile_pool(name="p2", bufs=1) as p2pool,
            tc.tile_pool(name="dramp", bufs=1, space="DRAM") as dpool,
            tc.tile_pool(name="gp", bufs=8) as gpool,
        ):
            # persistent result tiles; column k = n*NT + tc
            NK = NLOC * NT
            CH = 128          # gather chunk (elements); 512 B
            NCH = V // CH     # 8 chunks per row
            mx8 = rpool.tile([128, NK * 8], f32, tag="mx8", name="mx8")
            colmax = rpool.tile([128, NK * 8], f32, tag="colmax", name="colmax")
            c48 = rpool.tile([128, NK * 8], u32, tag="c48", name="c48")
            w8 = rpool.tile([128, NK * 8], u32, tag="w8", name="w8")
            se = rpool.tile([128, NK], f32, tag="se", name="se")

            # base_all[p, (n, tc)] = 512*p + 32*tc + 8*n: the 512B-chunk id of
            # row (t=16p+tc, n) is base + c8 (row id t*4+n, 8 chunks per row)
            base_all = rpool.tile([128, NLOC, NT], i32, tag="base_all", name="base_all")
            nc.gpsimd.iota(
                base_all[:], pattern=[[8, NLOC], [32, NT]], base=0,
                channel_multiplier=512,
            )

            # ---- phase 1 (groups of G tiles; each group's chunk-gather and
            # within-chunk argmax pipeline behind later groups' DMA/ACT) ----
            G = 8
            c4s = c48[:].rearrange("p (s e) -> p s e", e=8)[:, :, 0]
            base_flat = base_all[:].rearrange("p a b -> p (a b)")
            g32 = rpool.tile([128, NK], u32, tag="g32", name="g32")
            lg_flat = lg.ap().rearrange("t n (c e) -> (t n c) e", e=CH)

            for k0 in range(0, NK, G):
                for k in range(k0, k0 + G):
                    n, tch = divmod(k, NT)
                    xt = xpool.tile([128, V], f32, tag="x")
                    nc.sync.dma_start(out=xt[:], in_=lg_v[:, tch, n, :])
                    et = epool.tile([128, V], f32, tag="e")
                    nc.scalar.activation(
                        et[:], xt[:], AFT.Exp,
                        accum_out=se[:, k : k + 1],
                    )
                    # hierarchical x-domain max/argmax: 4 chunk-maxes, then
                    # top-8 of the slot, then the index of the max chunk
                    xv = xt[:].rearrange("p (c e) -> p c e", c=NCH)
                    nc.vector.reduce_max(
                        colmax[:, k * 8 : k * 8 + NCH], xv, axis=mybir.AxisListType.X
                    )
                    mxv = mx8[:, k * 8 : (k + 1) * 8]
                    cmv = colmax[:, k * 8 : (k + 1) * 8]
                    nc.vector.max(mxv, cmv)
                    nc.vector.max_index(c48[:, k * 8 : (k + 1) * 8], mxv, cmv)

                # per-group chunk ids, then a per-partition indirect gather of
                # each row's winning 512B chunk straight from DRAM
                nc.vector.scalar_tensor_tensor(
                    g32[:, k0 : k0 + G], c4s[:, k0 : k0 + G], 0,
                    base_flat[:, k0 : k0 + G], Alu.add, Alu.add,
                )
                for k in range(k0, k0 + G):
                    gt = gpool.tile([128, CH], f32, tag="g")
                    nc.gpsimd.indirect_dma_start(
                        gt[:],
                        None,
                        lg_flat,
                        bass.IndirectOffsetOnAxis(ap=g32[:, k : k + 1], axis=0),
                    )
                    nc.vector.max_index(
                        w8[:, k * 8 : (k + 1) * 8], mx8[:, k * 8 : (k + 1) * 8], gt[:]
                    )

            # ---- phase 1.5: batched epilogue + reshape to [n, t] rows ----
            ami_nt = p2pool.tile([16, T], f32, tag="ami_nt")
            logp_nt = p2pool.tile([16, T], f32, tag="logp_nt")
            lnse = rpool.tile([128, NK], f32, tag="lnse", name="lnse")
            nc.scalar.activation(lnse[:], se[:], AFT.Ln)
            mxs = mx8[:].rearrange("p (s e) -> p s e", e=8)[:, :, 0]
            logp = rpool.tile([128, NK], f32, tag="logp", name="logp")
            # logp = maxlogp = max(x) - ln(sum e^x)
            nc.vector.scalar_tensor_tensor(
                logp[:], mxs, 0.0, lnse[:], Alu.add, Alu.subtract
            )
            # argmax = c4*256 + w, emitted directly as f32
            amif = rpool.tile([128, NK], f32, tag="amif", name="amif")
            ws = w8[:].rearrange("p (s e) -> p s e", e=8)[:, :, 0]
            nc.vector.scalar_tensor_tensor(
                amif[:], c4s, float(CH), ws, Alu.mult, Alu.add
            )
            for n in range(NLOC):
                # [128, 16] (p-major, tc-minor) -> one row of 2048 (t = 16p+tc)
                nc.sync.dma_start(
                    out=ami_nt[n : n + 1, :], in_=amif[:, n * NT : (n + 1) * NT]
                )
                nc.sync.dma_start(
                    out=logp_nt[n : n + 1, :], in_=logp[:, n * NT : (n + 1) * NT]
                )

            # ---- phase 2 ----
            lens_sb = p2pool.tile([16, 1], f32, tag="lens_sb")
            nc.sync.dma_start(out=lens_sb[0:NLOC, :], in_=ll.ap())

            it = p2pool.tile([16, T], f32, tag="iota", name="it")
            nc.sync.dma_start(out=it[:], in_=io.ap())

            def t4(tag, dt=f32, w=T):
                return p2pool.tile([16, w], dt, tag=tag, name=tag)

            lm = t4("lm")
            nc.vector.tensor_scalar(lm[:NLOC], it[:NLOC], lens_sb[:NLOC, :], None, Alu.is_lt)

            nb = t4("nb")
            nc.vector.scalar_tensor_tensor(
                nb[:NLOC], ami_nt[:NLOC], float(BLANK), lm[:NLOC], Alu.not_equal, Alu.mult
            )

            neq = t4("neq")
            nc.vector.memset(neq[:NLOC, 0:1], 1.0)
            nc.vector.scalar_tensor_tensor(
                neq[:NLOC, 1:], ami_nt[:NLOC, 1:], 0.0, ami_nt[:NLOC, : T - 1],
                Alu.add, Alu.not_equal,
            )

            keep = t4("keep")
            olf = p2pool.tile([16, 1], f32, tag="olf")
            nc.vector.scalar_tensor_tensor(
                keep[:NLOC], nb[:NLOC], 0.0, neq[:NLOC], Alu.add, Alu.mult,
                accum_out=olf[:NLOC, :],
            )

            scrap = t4("scrap")
            mts = p2pool.tile([16, 1], f32, tag="mts")
            nc.vector.scalar_tensor_tensor(
                scrap[:NLOC], logp_nt[:NLOC], 0.0, lm[:NLOC], Alu.add, Alu.mult,
                accum_out=mts[:NLOC, :],
            )

            cum = t4("cum")
            nc.vector.tensor_tensor_scan(
                cum[:NLOC], keep[:NLOC], keep[:NLOC], 0.0, Alu.add, Alu.bypass
            )

            # safe_pos + 1 = keep * cum  (0 where dropped, pos+1 where kept)
            spp1 = t4("spp1")
            nc.vector.scalar_tensor_tensor(
                spp1[:NLOC], keep[:NLOC], 0.0, cum[:NLOC], Alu.add, Alu.mult
            )

            idx_hi = t4("idx_hi", i16)
            nc.gpsimd.memset(idx_hi[:, :], -1)
            nc.vector.tensor_scalar(idx_hi[:NLOC], spp1[:NLOC], 1025.0, None, Alu.subtract)

            mhi = t4("mhi")
            nc.vector.tensor_scalar(mhi[:NLOC], spp1[:NLOC], 1025.0, None, Alu.is_ge)
            tlo = t4("tlo")
            nc.vector.scalar_tensor_tensor(
                tlo[:NLOC], mhi[:NLOC], -2048.0, spp1[:NLOC], Alu.mult, Alu.add
            )
            idx_lo = t4("idx_lo", i16)
            nc.gpsimd.memset(idx_lo[:, :], -1)
            nc.vector.tensor_scalar(idx_lo[:NLOC], tlo[:NLOC], 1.0, None, Alu.subtract)

            dat16 = t4("dat16", i16)
            nc.gpsimd.memset(dat16[:, :], 0)
            nc.vector.tensor_copy(dat16[:NLOC], ami_nt[:NLOC])

            cmp16 = t4("cmp16", i16)
            nc.gpsimd.local_scatter(
                cmp16[:, : T // 2], dat16[:], idx_lo[:],
                channels=16, num_elems=T // 2, num_idxs=T,
            )
            nc.gpsimd.local_scatter(
                cmp16[:, T // 2 :], dat16[:], idx_hi[:],
                channels=16, num_elems=T // 2, num_idxs=T,
            )

            cmpf = t4("cmpf")
            nc.vector.tensor_copy(cmpf[:NLOC], cmp16[:NLOC])

            msel = t4("msel", i32)
            nc.vector.tensor_scalar(msel[:NLOC], it[:NLOC], olf[:NLOC, :], None, Alu.is_lt)

            pf = t4("pf")
            nc.vector.select(pf[:NLOC], msel[:NLOC], cmpf[:NLOC], ami_nt[:NLOC])

            pi = t4("pi", i32)
            nc.vector.tensor_copy(pi[:NLOC], pf[:NLOC])
            oli = p2pool.tile([16, 1], i32, tag="oli")
            nc.vector.tensor_copy(oli[:NLOC], olf[:NLOC, :])

            nc.sync.dma_start(out=paths_o.ap(), in_=pi[:NLOC, :])
            nc.sync.dma_start(out=mt_o.ap(), in_=mts[:NLOC, :])
            nc.sync.dma_start(out=ol_o.ap(), in_=oli[:NLOC, :])

    return nc


def _get_nc():
    if "nc" not in _BUILT:
        nc = build_nc()
        nc.finalize()
        _BUILT["nc"] = nc
    return _BUILT["nc"]


_IOTA = np.broadcast_to(np.arange(T, dtype=np.float32), (16, T)).copy()


def make_in_maps(logits, in_lens):
    logits = np.ascontiguousarray(np.asarray(logits, dtype=np.float32))
    lens = np.asarray(in_lens).astype(np.float32).reshape(N)
    in_maps = []
    for c in range(NCORES):
        sl = slice(NLOC * c, NLOC * (c + 1))
        in_maps.append(
            {
                "logits": np.ascontiguousarray(logits[:, sl, :]),
                "lens_f32": np.ascontiguousarray(lens[sl].reshape(NLOC, 1)),
                "iota_f32": _IOTA,
            }
        )
    return in_maps


def kernel(logits, in_lens):
    from concourse.bass_utils import run_bass_kernel_spmd

    nc = _get_nc()
    in_maps = make_in_maps(logits, in_lens)
    res = run_bass_kernel_spmd(nc, in_maps, core_ids=list(range(NCORES))).results

    mt = np.concatenate([np.asarray(r["max_total"]).reshape(NLOC) for r in res])
    ol = np.concatenate([np.asarray(r["out_lens"]).reshape(NLOC) for r in res])
    paths = np.concatenate(
        [np.asarray(r["paths"]).reshape(NLOC, T) for r in res], axis=0
    )
    return (
        mt.astype(np.float32),
        np.ascontiguousarray(paths.T).astype(np.int32),
        ol.astype(np.int32),
    )


# revision 33
# speedup vs baseline: 1.2892x; 1.0093x over previous
"""CTC greedy search Trainium2 kernel (8-core data parallel over batch).

Problem: logits (T=2048, N=32, V=1024) f32, in_lens (N,) int.
Returns (max_total f32 (N,), paths i32 (T, N), out_lens i32 (N,)).

Sharding: batch N split 4-per-core across 8 cores; everything else local.

Per-core algorithm:
  phase 1 (per [128, 1024] tile; rows are (n, t) pairs with t = 16*p + tc):
    - DMA tile in
    - ACT: exp(x) with accum -> sum_exp per row (raw exp is safe: |x| <= ~6)
    - DVE: max8 -> row max (top-8, we use [0]); max_index -> argmax (first occurrence)
  phase 1.5 (per n): maxlogp = max - ln(sum_exp); reshape [128,16] -> [1,2048]
    via cross-partition DMA so each n's t-sequence is one partition row.
  phase 2 (rows [4, 2048]): masks, dedup, cumsum (tensor_tensor_scan),
    compaction via two gpsimd local_scatter calls (dst halves of 1024, using
    the negative-index-is-ignored rule), merge with original argmax for the
    "undefined" tail, DMA out.
"""

import sys

if "/opt/trn_rl_repo" not in sys.path:
    sys.path.insert(0, "/opt/trn_rl_repo")

import numpy as np

T = 2048
N = 32
V = 1024
NCORES = 8
NLOC = N // NCORES  # 4
NT = 16             # t-chunks per n; t = 16*p + tc
BLANK = V - 1       # 1023

_BUILT = {}


def build_nc():
    import concourse.bass as bass
    import concourse.mybir as mybir
    from concourse.bacc import Bacc
    from concourse.tile import TileContext

    f32 = mybir.dt.float32
    i32 = mybir.dt.int32
    u32 = mybir.dt.uint32
    i16 = mybir.dt.int16
    Alu = mybir.AluOpType
    AFT = mybir.ActivationFunctionType

    nc = Bacc()
    lg = nc.declare_dram_parameter("logits", [T, NLOC, V], f32, isOutput=False)
    ll = nc.declare_dram_parameter("lens_f32", [NLOC, 1], f32, isOutput=False)
    iob = nc.declare_dram_parameter("iota_b", [128, 64], f32, isOutput=False)
    llb = nc.declare_dram_parameter("lens_b", [128, 1], f32, isOutput=False)
    paths_o = nc.declare_dram_parameter("paths", [NLOC, T], i32, isOutput=True)
    mt_o = nc.declare_dram_parameter("max_total", [NLOC, 1], f32, isOutput=True)
    ol_o = nc.declare_dram_parameter("out_lens", [NLOC, 1], i32, isOutput=True)
    import os as _os
    DEBUG = _os.environ.get("KDEBUG", "0") == "1"
    if DEBUG:
        dbg_c = nc.declare_dram_parameter("dbg_c", [16, T], i32, isOutput=True)
        dbg_il = nc.declare_dram_parameter("dbg_il", [16, T], i32, isOutput=True)
        dbg_ih = nc.declare_dram_parameter("dbg_ih", [16, T], i32, isOutput=True)
        dbg_dt = nc.declare_dram_parameter("dbg_dt", [16, T], i32, isOutput=True)

    # logits (t, n, v) viewed as [p, tc, n, v] with t = 16*p + tc
    lg_v = lg.ap().rearrange("(p s) n v -> p s n v", s=NT)

    with TileContext(nc) as tc_ctx:
        tc = tc_ctx
        with (
            tc.tile_pool(name="xp", bufs=8) as xpool,
            tc.tile_pool(name="ep", bufs=2, space="PSUM") as epool,
            tc.tile_pool(name="res", bufs=1) as rpool,
            tc.tile_pool(name="p2", bufs=1) as p2pool,
            tc.tile_pool(name="dramp", bufs=1, space="DRAM") as dpool,
            tc.tile_pool(name="gp", bufs=12) as gpool,
        ):
            # persistent result tiles; column k = n*NT + tc
            NK = NLOC * NT
            CH = 128          # gather chunk (elements); 512 B
            NCH = V // CH     # 8 chunks per row
            mx8 = rpool.tile([128, NK * 8], f32, tag="mx8", name="mx8")
            colmax = rpool.tile([128, NK * 8], f32, tag="colmax", name="colmax")
            c48 = rpool.tile([128, NK * 8], u32, tag="c48", name="c48")
            w8 = rpool.tile([128, NK * 8], u32, tag="w8", name="w8")
            se = rpool.tile([128, NK], f32, tag="se", name="se")

            # base_all[p, (n, tc)] = 512*p + 32*tc + 8*n: the 512B-chunk id of
            # row (t=16p+tc, n) is base + c8 (row id t*4+n, 8 chunks per row)
            base_all = rpool.tile([128, NLOC, NT], i32, tag="base_all", name="base_all")
            nc.gpsimd.iota(
                base_all[:], pattern=[[8, NLOC], [32, NT]], base=0,
                channel_multiplier=512,
            )

            # ---- phase 1 (groups of G tiles; each group's chunk-gather and
            # within-chunk argmax pipeline behind later groups' DMA/ACT) ----
            G = 8
            c4s = c48[:].rearrange("p (s e) -> p s e", e=8)[:, :, 0]
            base_flat = base_all[:].rearrange("p a b -> p (a b)")
            g32 = rpool.tile([128, NK], u32, tag="g32", name="g32")
            lg_flat = lg.ap().rearrange("t n (c e) -> (t n c) e", e=CH)

            for k0 in range(0, NK, G):
                for k in range(k0, k0 + G):
                    n, tch = divmod(k, NT)
                    xt = xpool.tile([128, V], f32, tag="x")
                    nc.sync.dma_start(out=xt[:], in_=lg_v[:, tch, n, :])
                    et = epool.tile([128, V], f32, tag="e")
                    nc.scalar.activation(
                        et[:], xt[:], AFT.Exp,
                        accum_out=se[:, k : k + 1],
                    )
                    # hierarchical x-domain max/argmax: 4 chunk-maxes, then
                    # top-8 of the slot, then the index of the max chunk
                    xv = xt[:].rearrange("p (c e) -> p c e", c=NCH)
                    nc.vector.reduce_max(
                        colmax[:, k * 8 : k * 8 + NCH], xv, axis=mybir.AxisListType.X
                    )
                    mxv = mx8[:, k * 8 : (k + 1) * 8]
                    cmv = colmax[:, k * 8 : (k + 1) * 8]
                    nc.vector.max(mxv, cmv)
                    nc.vector.max_index(c48[:, k * 8 : (k + 1) * 8], mxv, cmv)

                # per-group chunk ids, then a per-partition indirect gather of
                # each row's winning 512B chunk straight from DRAM
                nc.vector.scalar_tensor_tensor(
                    g32[:, k0 : k0 + G], c4s[:, k0 : k0 + G], 0,
                    base_flat[:, k0 : k0 + G], Alu.add, Alu.add,
                )
                for k in range(k0, k0 + G):
                    gt = gpool.tile([128, CH], f32, tag="g")
                    nc.gpsimd.indirect_dma_start(
                        gt[:],
                        None,
                        lg_flat,
                        bass.IndirectOffsetOnAxis(ap=g32[:, k : k + 1], axis=0),
                    )
                    nc.vector.max_index(
                        w8[:, k * 8 : (k + 1) * 8],
                        mx8[:, k * 8 : (k + 1) * 8],
                        gt[:],
                    )

            # ---- phase 1.5: batched epilogue; stage per-row argmax/maxlogp
            # to DRAM in (n, t) order, then reload in the blocked layout
            # [(n, psub), j] with t = psub*64 + j ----
            lnse = rpool.tile([128, NK], f32, tag="lnse", name="lnse")
            nc.scalar.activation(lnse[:], se[:], AFT.Ln)
            mxs = mx8[:].rearrange("p (s e) -> p s e", e=8)[:, :, 0]
            logp = rpool.tile([128, NK], f32, tag="logp", name="logp")
            # logp = maxlogp = max(x) - ln(sum e^x)
            nc.vector.scalar_tensor_tensor(
                logp[:], mxs, 0.0, lnse[:], Alu.add, Alu.subtract
            )
            # argmax = c8*128 + w, emitted directly as f32
            amif = rpool.tile([128, NK], f32, tag="amif", name="amif")
            ws = w8[:].rearrange("p (s e) -> p s e", e=8)[:, :, 0]
            nc.vector.scalar_tensor_tensor(
                amif[:], c4s, float(CH), ws, Alu.mult, Alu.add
            )
            st_ami = dpool.tile([NLOC, T], f32, tag="st_ami", name="st_ami")
            st_logp = dpool.tile([NLOC, T], f32, tag="st_logp", name="st_logp")
            for n in range(NLOC):
                # [128, 16] (p-major, tc-minor) -> one row of 2048 (t = 16p+tc)
                nc.sync.dma_start(
                    out=st_ami[n : n + 1, :], in_=amif[:, n * NT : (n + 1) * NT]
                )
                nc.sync.dma_start(
                    out=st_logp[n : n + 1, :], in_=logp[:, n * NT : (n + 1) * NT]
                )
            ami_b = p2pool.tile([128, 64], f32, tag="ami_b", name="ami_b")
            logp_b = p2pool.tile([128, 64], f32, tag="logp_b", name="logp_b")
            nc.sync.dma_start(
                out=ami_b[:], in_=st_ami[:].rearrange("n (q j) -> (n q) j", j=64)
            )
            nc.sync.dma_start(
                out=logp_b[:], in_=st_logp[:].rearrange("n (q j) -> (n q) j", j=64)
            )


            # ---- phase 2 (blocked [128, 64]; A-layout [16, T] only for the
            # per-row scan carries and the gpsimd scatter) ----
            iota_b = p2pool.tile([128, 64], f32, tag="iota_b", name="iota_b")
            nc.sync.dma_start(out=iota_b[:], in_=iob.ap())
            lens_sb = p2pool.tile([128, 1], f32, tag="lens_sb", name="lens_sb")
            nc.sync.dma_start(out=lens_sb[:], in_=llb.ap())

            def tb(tag, dt=f32):
                return p2pool.tile([128, 64], dt, tag=tag, name=tag)

            def t4(tag, dt=f32, w=T):
                return p2pool.tile([16, w], dt, tag=tag, name=tag)

            lm = tb("lm")
            nc.vector.tensor_scalar(lm[:], iota_b[:], lens_sb[:, :], None, Alu.is_lt)
            nb = tb("nb")
            nc.vector.scalar_tensor_tensor(
                nb[:], ami_b[:], float(BLANK), lm[:], Alu.not_equal, Alu.mult
            )

            # neq[t] = argmax[t] != argmax[t-1]; block-boundary column uses the
            # previous partition's last element; rows with t=0 are forced to 1
            prev0 = p2pool.tile([128, 1], f32, tag="prev0", name="prev0")
            # batch-row starts (t=0) always differ from their "previous" token;
            # write disjoint ranges so no cross-engine WAW ordering is needed
            for n in range(NLOC):
                nc.vector.memset(prev0[32 * n : 32 * n + 1, :], -1.0)
                nc.sync.dma_start(
                    out=prev0[32 * n + 1 : 32 * n + 32, :],
                    in_=ami_b[32 * n : 32 * n + 31, 63:64],
                )
            neq = tb("neq")
            nc.vector.scalar_tensor_tensor(
                neq[:, 1:], ami_b[:, 1:], 0.0, ami_b[:, :63], Alu.add, Alu.not_equal
            )
            nc.vector.scalar_tensor_tensor(
                neq[:, 0:1], ami_b[:, 0:1], 0.0, prev0[:], Alu.add, Alu.not_equal
            )

            keep = tb("keep")
            nc.vector.scalar_tensor_tensor(
                keep[:], nb[:], 0.0, neq[:], Alu.add, Alu.mult
            )
            # two-level scan: per-partition inclusive scan, then carry the 32
            # block totals per batch row through a tiny A-layout pass
            scb = tb("scb")
            nc.vector.tensor_tensor_scan(
                scb[:], keep[:], keep[:], 0.0, Alu.add, Alu.bypass
            )
            # blocked partial sums for max_total in the same style
            mtp = tb("mtp")
            mtpart = p2pool.tile([128, 1], f32, tag="mtpart", name="mtpart")
            nc.vector.scalar_tensor_tensor(
                mtp[:], logp_b[:], 0.0, lm[:], Alu.add, Alu.mult,
                accum_out=mtpart[:],
            )
            st_sm = dpool.tile([2, 128], f32, tag="st_sm", name="st_sm")
            nc.sync.dma_start(out=st_sm[0:1, :], in_=scb[:, 63:64])
            nc.sync.dma_start(out=st_sm[1:2, :], in_=mtpart[:])
            sm_a = p2pool.tile([16, 32], f32, tag="sm_a", name="sm_a")
            nc.sync.dma_start(
                out=sm_a[0:NLOC, :],
                in_=st_sm[0:1, :].rearrange("a (n q) -> (a n) q", q=32),
            )
            sm_m = p2pool.tile([16, 32], f32, tag="sm_m", name="sm_m")
            nc.sync.dma_start(
                out=sm_m[0:NLOC, :],
                in_=st_sm[1:2, :].rearrange("a (n q) -> (a n) q", q=32),
            )
            bts = p2pool.tile([16, 32], f32, tag="bts", name="bts")
            nc.vector.tensor_tensor_scan(
                bts[0:4, :], sm_a[0:4, :], sm_a[0:4, :], 0.0,
                Alu.add, Alu.bypass,
            )
            mts = p2pool.tile([16, 1], f32, tag="mts", name="mts")
            nc.vector.reduce_sum(
                mts[0:4, :], sm_m[0:4, :], axis=mybir.AxisListType.X
            )
            # exclusive prefix = inclusive - own; out_len = inclusive[31]
            exc = p2pool.tile([16, 32], f32, tag="exc", name="exc")
            nc.vector.scalar_tensor_tensor(
                exc[0:4, :], bts[0:4, :], 0.0, sm_a[0:4, :],
                Alu.add, Alu.subtract,
            )
            olf = p2pool.tile([16, 1], f32, tag="olf", name="olf")
            nc.vector.tensor_copy(olf[0:4, :], bts[0:4, 31:32])
            # bounce the carries back to blocked [128, 1]
            st_c = dpool.tile([1, 128], f32, tag="st_c", name="st_c")
            nc.sync.dma_start(
                out=st_c[0:1, :].rearrange("a (n q) -> (a n) q", q=32),
                in_=exc[0:4, :],
            )
            carry = p2pool.tile([128, 1], f32, tag="carry", name="carry")
            nc.sync.dma_start(out=carry[:], in_=st_c[0:1, :])
            cum = tb("cum")
            nc.vector.tensor_scalar(cum[:], scb[:], carry[:, :], None, Alu.add)

            # safe_pos + 1 = keep * cum  (0 where dropped, pos+1 where kept)
            spp1 = tb("spp1")
            nc.vector.scalar_tensor_tensor(
                spp1[:], keep[:], 0.0, cum[:], Alu.add, Alu.mult
            )
            idx_hi_b = tb("idx_hi_b", i16)
            nc.vector.tensor_scalar(idx_hi_b[:], spp1[:], 1025.0, None, Alu.subtract)
            mhi = tb("mhi")
            nc.vector.tensor_scalar(mhi[:], spp1[:], 1025.0, None, Alu.is_ge)
            tlo = tb("tlo")
            nc.vector.scalar_tensor_tensor(
                tlo[:], mhi[:], -2048.0, spp1[:], Alu.mult, Alu.add
            )
            idx_lo_b = tb("idx_lo_b", i16)
            nc.vector.tensor_scalar(idx_lo_b[:], tlo[:], 1.0, None, Alu.subtract)
            # scatter argmax+1 so an untouched (zeroed) slot is identifiable
            dat16_b = tb("dat16_b", i16)
            nc.vector.tensor_scalar(dat16_b[:], ami_b[:], 1.0, None, Alu.add)

            # reshape to [16, T] rows for the per-partition local_scatter
            idx_hi = t4("idx_hi", i16)
            nc.gpsimd.memset(idx_hi[:, :], -1)
            idx_lo = t4("idx_lo", i16)
            nc.gpsimd.memset(idx_lo[:, :], -1)
            dat16 = t4("dat16", i16)
            nc.gpsimd.memset(dat16[:, :], 0)
            nc.sync.dma_start(out=idx_hi[0:NLOC, :], in_=idx_hi_b[:])
            nc.sync.dma_start(out=idx_lo[0:NLOC, :], in_=idx_lo_b[:])
            nc.sync.dma_start(out=dat16[0:NLOC, :], in_=dat16_b[:])

            cmp16 = t4("cmp16", i16)
            nc.gpsimd.local_scatter(
                cmp16[:, : T // 2], dat16[:], idx_lo[:],
                channels=16, num_elems=T // 2, num_idxs=T,
            )
            nc.gpsimd.local_scatter(
                cmp16[:, T // 2 :], dat16[:], idx_hi[:],
                channels=16, num_elems=T // 2, num_idxs=T,
            )

            if DEBUG:
                for nm, tile_ in (("dbg_c", cmp16), ("dbg_il", idx_lo), ("dbg_ih", idx_hi), ("dbg_dt", dat16)):
                    cnv = p2pool.tile([16, T], i32, tag="cnv_"+nm, name="cnv_"+nm)
                    nc.vector.tensor_copy(cnv[:], tile_[:])
                    nc.sync.dma_start(out={"dbg_c": dbg_c, "dbg_il": dbg_il, "dbg_ih": dbg_ih, "dbg_dt": dbg_dt}[nm].ap(), in_=cnv[:])
            cmp_b = tb("cmp_b", i16)
            nc.sync.dma_start(out=cmp_b[:], in_=cmp16[0:NLOC, :])
            cmpf = tb("cmpf")
            nc.vector.tensor_scalar(cmpf[:], cmp_b[:], 1.0, None, Alu.subtract)
            msel = tb("msel", i32)
            nc.vector.tensor_scalar(msel[:], cmpf[:], 0.0, None, Alu.is_ge)
            pf = tb("pf")
            nc.vector.select(pf[:], msel[:], cmpf[:], ami_b[:])
            pi = tb("pi", i32)
            nc.vector.tensor_copy(pi[:], pf[:])
            oli = p2pool.tile([16, 1], i32, tag="oli", name="oli")
            nc.vector.tensor_copy(oli[0:NLOC, :], olf[0:NLOC, :])

            nc.sync.dma_start(
                out=paths_o.ap().rearrange("n (q j) -> (n q) j", j=64), in_=pi[:]
            )
            nc.sync.dma_start(out=mt_o.ap(), in_=mts[0:NLOC, :])
            nc.sync.dma_start(out=ol_o.ap(), in_=oli[0:NLOC, :])

    return nc


def _get_nc():
    if "nc" not in _BUILT:
        nc = build_nc()
        nc.finalize()
        _BUILT["nc"] = nc
    return _BUILT["nc"]


_IOTA_B = (
    (np.arange(128)[:, None] % 32) * 64 + np.arange(64)[None, :]
).astype(np.float32)


def make_in_maps(logits, in_lens):
    logits = np.ascontiguousarray(np.asarray(logits, dtype=np.float32))
    lens = np.asarray(in_lens).astype(np.float32).reshape(N)
    in_maps = []
    for c in range(NCORES):
        sl = slice(NLOC * c, NLOC * (c + 1))
        in_maps.append(
            {
                "logits": np.ascontiguousarray(logits[:, sl, :]),
                "lens_f32": np.ascontiguousarray(lens[sl].reshape(NLOC, 1)),
                "iota_b": _IOTA_B,
                "lens_b": np.ascontiguousarray(
                    np.repeat(lens[sl], 32).reshape(128, 1)
                ),
            }
        )
    return in_maps


def kernel(logits, in_lens):
    from concourse.bass_utils import run_bass_kernel_spmd

    nc = _get_nc()
    in_maps = make_in_maps(logits, in_lens)
    res = run_bass_kernel_spmd(nc, in_maps, core_ids=list(range(NCORES))).results

    mt = np.concatenate([np.asarray(r["max_total"]).reshape(NLOC) for r in res])
    ol = np.concatenate([np.asarray(r["out_lens"]).reshape(NLOC) for r in res])
    paths = np.concatenate(
        [np.asarray(r["paths"]).reshape(NLOC, T) for r in res], axis=0
    )
    return (
        mt.astype(np.float32),
        np.ascontiguousarray(paths.T).astype(np.int32),
        ol.astype(np.int32),
    )


# revision 45
# speedup vs baseline: 1.3386x; 1.0383x over previous
"""CTC greedy search Trainium2 kernel (8-core data parallel over batch).

Problem: logits (T=2048, N=32, V=1024) f32, in_lens (N,) int.
Returns (max_total f32 (N,), paths i32 (T, N), out_lens i32 (N,)).

Sharding: batch N split 4-per-core across 8 cores; everything else local.

Per-core algorithm:
  phase 1 (per [128, 1024] tile; rows are (n, t) pairs with t = 16*p + tc):
    - DMA tile in
    - ACT: exp(x) with accum -> sum_exp per row (raw exp is safe: |x| <= ~6)
    - DVE: max8 -> row max (top-8, we use [0]); max_index -> argmax (first occurrence)
  phase 1.5 (per n): maxlogp = max - ln(sum_exp); reshape [128,16] -> [1,2048]
    via cross-partition DMA so each n's t-sequence is one partition row.
  phase 2 (rows [4, 2048]): masks, dedup, cumsum (tensor_tensor_scan),
    compaction via two gpsimd local_scatter calls (dst halves of 1024, using
    the negative-index-is-ignored rule), merge with original argmax for the
    "undefined" tail, DMA out.
"""

import sys

if "/opt/trn_rl_repo" not in sys.path:
    sys.path.insert(0, "/opt/trn_rl_repo")

import numpy as np

T = 2048
N = 32
V = 1024
NCORES = 8
NLOC = N // NCORES  # 4
NT = 16             # t-chunks per n; t = 16*p + tc
BLANK = V - 1       # 1023

_BUILT = {}


def build_nc():
    import concourse.bass as bass
    import concourse.mybir as mybir
    from concourse.bacc import Bacc
    from concourse.tile import TileContext

    f32 = mybir.dt.float32
    i32 = mybir.dt.int32
    u32 = mybir.dt.uint32
    i16 = mybir.dt.int16
    Alu = mybir.AluOpType
    AFT = mybir.ActivationFunctionType

    nc = Bacc()
    lg = nc.declare_dram_parameter("logits", [T, NLOC, V], f32, isOutput=False)
    ll = nc.declare_dram_parameter("lens_f32", [NLOC, 1], f32, isOutput=False)
    iob = nc.declare_dram_parameter("iota_b", [128, 64], f32, isOutput=False)
    llb = nc.declare_dram_parameter("lens_b", [128, 1], f32, isOutput=False)
    paths_o = nc.declare_dram_parameter("paths", [NLOC, T], i32, isOutput=True)
    mt_o = nc.declare_dram_parameter("max_total", [NLOC, 1], f32, isOutput=True)
    ol_o = nc.declare_dram_parameter("out_lens", [NLOC, 1], i32, isOutput=True)
    import os as _os
    DEBUG = _os.environ.get("KDEBUG", "0") == "1"
    if DEBUG:
        dbg_c = nc.declare_dram_parameter("dbg_c", [16, T], i32, isOutput=True)
        dbg_il = nc.declare_dram_parameter("dbg_il", [16, T], i32, isOutput=True)
        dbg_ih = nc.declare_dram_parameter("dbg_ih", [16, T], i32, isOutput=True)
        dbg_dt = nc.declare_dram_parameter("dbg_dt", [16, T], i32, isOutput=True)

    # logits (t, n, v) viewed as [p, tc, n, v] with t = 16*p + tc
    lg_v = lg.ap().rearrange("(p s) n v -> p s n v", s=NT)

    with TileContext(nc) as tc_ctx:
        tc = tc_ctx
        with (
            tc.tile_pool(name="xp", bufs=5) as xpool,
            tc.tile_pool(name="ep", bufs=2, space="PSUM") as epool,
            tc.tile_pool(name="res", bufs=1) as rpool,
            tc.tile_pool(name="p2", bufs=1) as p2pool,
            tc.tile_pool(name="dramp", bufs=1, space="DRAM") as dpool,
            tc.tile_pool(name="gp", bufs=12) as gpool,
        ):
            # persistent result tiles; column k = n*NT + tc
            NK = NLOC * NT
            CH = 128          # gather chunk (elements); 512 B
            NCH = V // CH     # 8 chunks per row
            mx8 = rpool.tile([128, NK * 8], f32, tag="mx8", name="mx8")
            colmax = rpool.tile([128, NK * 8], f32, tag="colmax", name="colmax")
            c48 = rpool.tile([128, NK * 8], u32, tag="c48", name="c48")
            w8 = rpool.tile([128, NK * 8], u32, tag="w8", name="w8")
            se = rpool.tile([128, NK], f32, tag="se", name="se")

            # base_all[p, (n, tc)] = 512*p + 32*tc + 8*n: the 512B-chunk id of
            # row (t=16p+tc, n) is base + c8 (row id t*4+n, 8 chunks per row)
            base_all = rpool.tile([128, NLOC, NT], i32, tag="base_all", name="base_all")
            nc.gpsimd.iota(
                base_all[:], pattern=[[8, NLOC], [32, NT]], base=0,
                channel_multiplier=512,
            )

            # ---- phase 1 (groups of G tiles; each group's chunk-gather and
            # within-chunk argmax pipeline behind later groups' DMA/ACT) ----
            G = 8
            c4s = c48[:].rearrange("p (s e) -> p s e", e=8)[:, :, 0]
            base_flat = base_all[:].rearrange("p a b -> p (a b)")
            g32 = rpool.tile([128, NK], u32, tag="g32", name="g32")
            lg_flat = lg.ap().rearrange("t n (c e) -> (t n c) e", e=CH)

            # phase-1.5/2 persistent tiles, created up front so per-n work
            # can be emitted inside the main loop
            lnse = rpool.tile([128, NK], f32, tag="lnse", name="lnse")
            logp = rpool.tile([128, NK], f32, tag="logp", name="logp")
            amif = rpool.tile([128, NK], f32, tag="amif", name="amif")
            mxs = mx8[:].rearrange("p (s e) -> p s e", e=8)[:, :, 0]
            ws = w8[:].rearrange("p (s e) -> p s e", e=8)[:, :, 0]
            st_ami = dpool.tile([NLOC, T], f32, tag="st_ami", name="st_ami")
            st_logp = dpool.tile([NLOC, T], f32, tag="st_logp", name="st_logp")
            ami_b = p2pool.tile([128, 64], f32, tag="ami_b", name="ami_b")
            logp_b = p2pool.tile([128, 64], f32, tag="logp_b", name="logp_b")
            iota_b = p2pool.tile([128, 64], f32, tag="iota_b", name="iota_b")
            nc.sync.dma_start(out=iota_b[:], in_=iob.ap())
            lens_sb = p2pool.tile([128, 1], f32, tag="lens_sb", name="lens_sb")
            nc.sync.dma_start(out=lens_sb[:], in_=llb.ap())

            def tb(tag, dt=f32):
                return p2pool.tile([128, 64], dt, tag=tag, name=tag)

            def t4(tag, dt=f32, w=T):
                return p2pool.tile([16, w], dt, tag=tag, name=tag)

            lm = tb("lm")
            nb = tb("nb")
            prev0 = p2pool.tile([128, 1], f32, tag="prev0", name="prev0")
            neq = tb("neq")
            keep = tb("keep")
            scb = tb("scb")
            mtp = tb("mtp")
            mtpart = p2pool.tile([128, 1], f32, tag="mtpart", name="mtpart")

            for k0 in range(0, NK, G):
                for k in range(k0, k0 + G):
                    n, tch = divmod(k, NT)
                    xt = xpool.tile([128, V], f32, tag="x")
                    nc.sync.dma_start(out=xt[:], in_=lg_v[:, tch, n, :])
                    et = epool.tile([128, V], f32, tag="e")
                    nc.scalar.activation(
                        et[:], xt[:], AFT.Exp,
                        accum_out=se[:, k : k + 1],
                    )
                    # hierarchical x-domain max/argmax: 4 chunk-maxes, then
                    # top-8 of the slot, then the index of the max chunk
                    xv = xt[:].rearrange("p (c e) -> p c e", c=NCH)
                    nc.vector.reduce_max(
                        colmax[:, k * 8 : k * 8 + NCH], xv, axis=mybir.AxisListType.X
                    )
                    mxv = mx8[:, k * 8 : (k + 1) * 8]
                    cmv = colmax[:, k * 8 : (k + 1) * 8]
                    nc.vector.max(mxv, cmv)
                    nc.vector.max_index(c48[:, k * 8 : (k + 1) * 8], mxv, cmv)

                # per-group chunk ids, then a per-partition indirect gather of
                # each row's winning 512B chunk straight from DRAM
                nc.vector.scalar_tensor_tensor(
                    g32[:, k0 : k0 + G], c4s[:, k0 : k0 + G], 0,
                    base_flat[:, k0 : k0 + G], Alu.add, Alu.add,
                )
                for k in range(k0, k0 + G):
                    gt = gpool.tile([128, CH], f32, tag="g")
                    nc.gpsimd.indirect_dma_start(
                        gt[:],
                        None,
                        lg_flat,
                        bass.IndirectOffsetOnAxis(ap=g32[:, k : k + 1], axis=0),
                    )
                    nc.vector.max_index(
                        w8[:, k * 8 : (k + 1) * 8],
                        mx8[:, k * 8 : (k + 1) * 8],
                        gt[:],
                    )

            # ---- phase 1.5 (batched): maxlogp, argmax, staging, blocked
            # reload, masks, dedup, per-partition scan ----
            nc.scalar.activation(lnse[:], se[:], AFT.Ln)
            nc.vector.scalar_tensor_tensor(
                logp[:], mxs, 0.0, lnse[:], Alu.add, Alu.subtract
            )
            nc.vector.scalar_tensor_tensor(
                amif[:], c4s, float(CH), ws, Alu.mult, Alu.add
            )
            for n in range(NLOC):
                nc.sync.dma_start(
                    out=st_ami[n : n + 1, :], in_=amif[:, n * NT : (n + 1) * NT]
                )
                nc.sync.dma_start(
                    out=st_logp[n : n + 1, :], in_=logp[:, n * NT : (n + 1) * NT]
                )
            nc.sync.dma_start(
                out=ami_b[:], in_=st_ami[:].rearrange("n (q j) -> (n q) j", j=64)
            )
            nc.sync.dma_start(
                out=logp_b[:], in_=st_logp[:].rearrange("n (q j) -> (n q) j", j=64)
            )
            nc.vector.tensor_scalar(lm[:], iota_b[:], lens_sb[:, :], None, Alu.is_lt)
            nc.vector.scalar_tensor_tensor(
                nb[:], ami_b[:], float(BLANK), lm[:], Alu.not_equal, Alu.mult
            )
            for n in range(NLOC):
                nc.vector.memset(prev0[32 * n : 32 * n + 1, :], -1.0)
                nc.sync.dma_start(
                    out=prev0[32 * n + 1 : 32 * n + 32, :],
                    in_=ami_b[32 * n : 32 * n + 31, 63:64],
                )
            nc.vector.scalar_tensor_tensor(
                neq[:, 1:], ami_b[:, 1:], 0.0, ami_b[:, :63], Alu.add, Alu.not_equal
            )
            nc.vector.scalar_tensor_tensor(
                neq[:, 0:1], ami_b[:, 0:1], 0.0, prev0[:], Alu.add, Alu.not_equal
            )
            nc.vector.scalar_tensor_tensor(
                keep[:], nb[:], 0.0, neq[:], Alu.add, Alu.mult
            )
            nc.vector.tensor_tensor_scan(
                scb[:], keep[:], keep[:], 0.0, Alu.add, Alu.bypass
            )
            nc.vector.scalar_tensor_tensor(
                mtp[:], logp_b[:], 0.0, lm[:], Alu.add, Alu.mult,
                accum_out=mtpart[:],
            )
            # [128,1] -> [4,32] direct SBUF->SBUF (flatten orders align)
            sm_a = p2pool.tile([16, 32], f32, tag="sm_a", name="sm_a")
            nc.sync.dma_start(out=sm_a[0:NLOC, :], in_=scb[:, 63:64])
            sm_m = p2pool.tile([16, 32], f32, tag="sm_m", name="sm_m")
            nc.sync.dma_start(out=sm_m[0:NLOC, :], in_=mtpart[:])
            bts = p2pool.tile([16, 32], f32, tag="bts", name="bts")
            nc.vector.tensor_tensor_scan(
                bts[0:4, :], sm_a[0:4, :], sm_a[0:4, :], 0.0,
                Alu.add, Alu.bypass,
            )
            mts = p2pool.tile([16, 1], f32, tag="mts", name="mts")
            nc.vector.reduce_sum(
                mts[0:4, :], sm_m[0:4, :], axis=mybir.AxisListType.X
            )
            # exclusive prefix = inclusive - own; out_len = inclusive[31]
            exc = p2pool.tile([16, 32], f32, tag="exc", name="exc")
            nc.vector.scalar_tensor_tensor(
                exc[0:4, :], bts[0:4, :], 0.0, sm_a[0:4, :],
                Alu.add, Alu.subtract,
            )
            olf = p2pool.tile([16, 1], f32, tag="olf", name="olf")
            nc.vector.tensor_copy(olf[0:4, :], bts[0:4, 31:32])
            carry = p2pool.tile([128, 1], f32, tag="carry", name="carry")
            nc.sync.dma_start(out=carry[:], in_=exc[0:4, :])
            # safe_pos + 1 = keep * (scan + carry): 0 where dropped
            spp1 = tb("spp1")
            nc.vector.scalar_tensor_tensor(
                spp1[:], scb[:], carry[:, :], keep[:], Alu.add, Alu.mult
            )
            idx_hi_b = tb("idx_hi_b", i16)
            nc.vector.tensor_scalar(idx_hi_b[:], spp1[:], 1025.0, None, Alu.subtract)
            mhi = tb("mhi")
            nc.vector.tensor_scalar(mhi[:], spp1[:], 1025.0, None, Alu.is_ge)
            tlo = tb("tlo")
            nc.vector.scalar_tensor_tensor(
                tlo[:], mhi[:], -2048.0, spp1[:], Alu.mult, Alu.add
            )
            idx_lo_b = tb("idx_lo_b", i16)
            nc.vector.tensor_scalar(idx_lo_b[:], tlo[:], 1.0, None, Alu.subtract)
            # scatter argmax+1 so an untouched (zeroed) slot is identifiable
            dat16_b = tb("dat16_b", i16)
            nc.vector.tensor_scalar(dat16_b[:], ami_b[:], 1.0, None, Alu.add)

            # reshape to [16, T] rows for the per-partition local_scatter
            idx_hi = t4("idx_hi", i16)
            nc.gpsimd.memset(idx_hi[:, :], -1)
            idx_lo = t4("idx_lo", i16)
            nc.gpsimd.memset(idx_lo[:, :], -1)
            dat16 = t4("dat16", i16)
            nc.gpsimd.memset(dat16[:, :], 0)
            nc.sync.dma_start(out=idx_hi[0:NLOC, :], in_=idx_hi_b[:])
            nc.sync.dma_start(out=idx_lo[0:NLOC, :], in_=idx_lo_b[:])
            nc.sync.dma_start(out=dat16[0:NLOC, :], in_=dat16_b[:])

            cmp16 = t4("cmp16", i16)
            nc.gpsimd.local_scatter(
                cmp16[:, : T // 2], dat16[:], idx_lo[:],
                channels=16, num_elems=T // 2, num_idxs=T,
            )
            nc.gpsimd.local_scatter(
                cmp16[:, T // 2 :], dat16[:], idx_hi[:],
                channels=16, num_elems=T // 2, num_idxs=T,
            )

            if DEBUG:
                for nm, tile_ in (("dbg_c", cmp16), ("dbg_il", idx_lo), ("dbg_ih", idx_hi), ("dbg_dt", dat16)):
                    cnv = p2pool.tile([16, T], i32, tag="cnv_"+nm, name="cnv_"+nm)
                    nc.vector.tensor_copy(cnv[:], tile_[:])
                    nc.sync.dma_start(out={"dbg_c": dbg_c, "dbg_il": dbg_il, "dbg_ih": dbg_ih, "dbg_dt": dbg_dt}[nm].ap(), in_=cnv[:])
            cmp_b = tb("cmp_b", i16)
            nc.sync.dma_start(out=cmp_b[:], in_=cmp16[0:NLOC, :])
            cmpf = tb("cmpf")
            nc.vector.tensor_scalar(cmpf[:], cmp_b[:], 1.0, None, Alu.subtract)
            msel = tb("msel", i32)
            nc.vector.tensor_scalar(msel[:], cmpf[:], 0.0, None, Alu.is_ge)
            pf = tb("pf")
            nc.vector.select(pf[:], msel[:], cmpf[:], ami_b[:])
            pi = tb("pi", i32)
            nc.vector.tensor_copy(pi[:], pf[:])
            oli = p2pool.tile([16, 1], i32, tag="oli", name="oli")
            nc.vector.tensor_copy(oli[0:NLOC, :], olf[0:NLOC, :])

            nc.sync.dma_start(
                out=paths_o.ap().rearrange("n (q j) -> (n q) j", j=64), in_=pi[:]
            )
            nc.sync.dma_start(out=mt_o.ap(), in_=mts[0:NLOC, :])
            nc.sync.dma_start(out=ol_o.ap(), in_=oli[0:NLOC, :])

    return nc


def _get_nc():
    if "nc" not in _BUILT:
        nc = build_nc()
        nc.finalize()
        _BUILT["nc"] = nc
    return _BUILT["nc"]


_IOTA_B = (
    (np.arange(128)[:, None] % 32) * 64 + np.arange(64)[None, :]
).astype(np.float32)


def make_in_maps(logits, in_lens):
    logits = np.ascontiguousarray(np.asarray(logits, dtype=np.float32))
    lens = np.asarray(in_lens).astype(np.float32).reshape(N)
    in_maps = []
    for c in range(NCORES):
        sl = slice(NLOC * c, NLOC * (c + 1))
        in_maps.append(
            {
                "logits": np.ascontiguousarray(logits[:, sl, :]),
                "lens_f32": np.ascontiguousarray(lens[sl].reshape(NLOC, 1)),
                "iota_b": _IOTA_B,
                "lens_b": np.ascontiguousarray(
                    np.repeat(lens[sl], 32).reshape(128, 1)
                ),
            }
        )
    return in_maps


def kernel(logits, in_lens):
    from concourse.bass_utils import run_bass_kernel_spmd

    nc = _get_nc()
    in_maps = make_in_maps(logits, in_lens)
    res = run_bass_kernel_spmd(nc, in_maps, core_ids=list(range(NCORES))).results

    mt = np.concatenate([np.asarray(r["max_total"]).reshape(NLOC) for r in res])
    ol = np.concatenate([np.asarray(r["out_lens"]).reshape(NLOC) for r in res])
    paths = np.concatenate(
        [np.asarray(r["paths"]).reshape(NLOC, T) for r in res], axis=0
    )
    return (
        mt.astype(np.float32),
        np.ascontiguousarray(paths.T).astype(np.int32),
        ol.astype(np.int32),
    )


# revision 49
# speedup vs baseline: 1.3821x; 1.0325x over previous
"""CTC greedy search Trainium2 kernel (8-core data parallel over batch).

Problem: logits (T=2048, N=32, V=1024) f32, in_lens (N,) int.
Returns (max_total f32 (N,), paths i32 (T, N), out_lens i32 (N,)).

Sharding: batch N split 4-per-core across 8 cores; everything else local.

Per-core algorithm:
  phase 1 (per [128, 1024] tile; rows are (n, t) pairs with t = 16*p + tc):
    - DMA tile in
    - ACT: exp(x) with accum -> sum_exp per row (raw exp is safe: |x| <= ~6)
    - DVE: max8 -> row max (top-8, we use [0]); max_index -> argmax (first occurrence)
  phase 1.5 (per n): maxlogp = max - ln(sum_exp); reshape [128,16] -> [1,2048]
    via cross-partition DMA so each n's t-sequence is one partition row.
  phase 2 (rows [4, 2048]): masks, dedup, cumsum (tensor_tensor_scan),
    compaction via two gpsimd local_scatter calls (dst halves of 1024, using
    the negative-index-is-ignored rule), merge with original argmax for the
    "undefined" tail, DMA out.
"""

import sys

if "/opt/trn_rl_repo" not in sys.path:
    sys.path.insert(0, "/opt/trn_rl_repo")

import numpy as np

T = 2048
N = 32
V = 1024
NCORES = 8
NLOC = N // NCORES  # 4
NT = 16             # t-chunks per n; t = 16*p + tc
BLANK = V - 1       # 1023

_BUILT = {}


def build_nc():
    import concourse.bass as bass
    import concourse.mybir as mybir
    from concourse.bacc import Bacc
    from concourse.tile import TileContext

    f32 = mybir.dt.float32
    i32 = mybir.dt.int32
    u32 = mybir.dt.uint32
    i16 = mybir.dt.int16
    Alu = mybir.AluOpType
    AFT = mybir.ActivationFunctionType

    nc = Bacc()
    lg = nc.declare_dram_parameter("logits", [T, NLOC, V], f32, isOutput=False)
    ll = nc.declare_dram_parameter("lens_f32", [NLOC, 1], f32, isOutput=False)
    iob = nc.declare_dram_parameter("iota_b", [128, 64], f32, isOutput=False)
    llb = nc.declare_dram_parameter("lens_b", [128, 1], f32, isOutput=False)
    paths_o = nc.declare_dram_parameter("paths", [NLOC, T], i32, isOutput=True)
    mt_o = nc.declare_dram_parameter("max_total", [NLOC, 1], f32, isOutput=True)
    ol_o = nc.declare_dram_parameter("out_lens", [NLOC, 1], i32, isOutput=True)
    import os as _os
    DEBUG = _os.environ.get("KDEBUG", "0") == "1"
    if DEBUG:
        dbg_c = nc.declare_dram_parameter("dbg_c", [16, T], i32, isOutput=True)
        dbg_il = nc.declare_dram_parameter("dbg_il", [16, T], i32, isOutput=True)
        dbg_ih = nc.declare_dram_parameter("dbg_ih", [16, T], i32, isOutput=True)
        dbg_dt = nc.declare_dram_parameter("dbg_dt", [16, T], i32, isOutput=True)

    # logits (t, n, v) viewed as [p, tc, n, v] with t = 16*p + tc
    lg_v = lg.ap().rearrange("(p s) n v -> p s n v", s=NT)

    with TileContext(nc) as tc_ctx:
        tc = tc_ctx
        with (
            tc.tile_pool(name="xp", bufs=5) as xpool,
            tc.tile_pool(name="ep", bufs=2, space="PSUM") as epool,
            tc.tile_pool(name="res", bufs=1) as rpool,
            tc.tile_pool(name="p2", bufs=1) as p2pool,
            tc.tile_pool(name="dramp", bufs=1, space="DRAM") as dpool,
            tc.tile_pool(name="gp", bufs=12) as gpool,
        ):
            # persistent result tiles; column k = n*NT + tc
            NK = NLOC * NT
            CH = 128          # gather chunk (elements); 512 B
            NCH = V // CH     # 8 chunks per row
            mx8 = rpool.tile([128, NK * 8], f32, tag="mx8", name="mx8")
            colmax = rpool.tile([128, NK * 8], f32, tag="colmax", name="colmax")
            c48 = rpool.tile([128, NK * 8], u32, tag="c48", name="c48")
            w8 = rpool.tile([128, NK * 8], u32, tag="w8", name="w8")
            se = rpool.tile([128, NK], f32, tag="se", name="se")

            # base_all[p, (n, tc)] = 512*p + 32*tc + 8*n: the 512B-chunk id of
            # row (t=16p+tc, n) is base + c8 (row id t*4+n, 8 chunks per row)
            base_all = rpool.tile([128, NLOC, NT], i32, tag="base_all", name="base_all")
            nc.gpsimd.iota(
                base_all[:], pattern=[[8, NLOC], [32, NT]], base=0,
                channel_multiplier=512,
            )

            # ---- phase 1 (groups of G tiles; each group's chunk-gather and
            # within-chunk argmax pipeline behind later groups' DMA/ACT) ----
            G = 8
            c4s = c48[:].rearrange("p (s e) -> p s e", e=8)[:, :, 0]
            base_flat = base_all[:].rearrange("p a b -> p (a b)")
            g32 = rpool.tile([128, NK], u32, tag="g32", name="g32")
            lg_flat = lg.ap().rearrange("t n (c e) -> (t n c) e", e=CH)

            # phase-1.5/2 persistent tiles, created up front so per-n work
            # can be emitted inside the main loop
            lnse = rpool.tile([128, NK], f32, tag="lnse", name="lnse")
            logp = rpool.tile([128, NK], f32, tag="logp", name="logp")
            amif = rpool.tile([128, NK], f32, tag="amif", name="amif")
            mxs = mx8[:].rearrange("p (s e) -> p s e", e=8)[:, :, 0]
            ws = w8[:].rearrange("p (s e) -> p s e", e=8)[:, :, 0]
            st_ami = dpool.tile([NLOC, T], f32, tag="st_ami", name="st_ami")
            st_logp = dpool.tile([NLOC, T], f32, tag="st_logp", name="st_logp")
            ami_b = p2pool.tile([128, 64], f32, tag="ami_b", name="ami_b")
            logp_b = p2pool.tile([128, 64], f32, tag="logp_b", name="logp_b")
            iota_b = p2pool.tile([128, 64], f32, tag="iota_b", name="iota_b")
            nc.sync.dma_start(out=iota_b[:], in_=iob.ap())
            lens_sb = p2pool.tile([128, 1], f32, tag="lens_sb", name="lens_sb")
            nc.sync.dma_start(out=lens_sb[:], in_=llb.ap())

            def tb(tag, dt=f32):
                return p2pool.tile([128, 64], dt, tag=tag, name=tag)

            def t4(tag, dt=f32, w=T):
                return p2pool.tile([16, w], dt, tag=tag, name=tag)

            lm = tb("lm")
            nb = tb("nb")
            prev0 = p2pool.tile([128, 1], f32, tag="prev0", name="prev0")
            neq = tb("neq")
            keep = tb("keep")
            scb = tb("scb")
            mtp = tb("mtp")
            mtpart = p2pool.tile([128, 1], f32, tag="mtpart", name="mtpart")

            for k0 in range(0, NK, G):
                for k in range(k0, k0 + G):
                    n, tch = divmod(k, NT)
                    xt = xpool.tile([128, V], f32, tag="x")
                    nc.sync.dma_start(out=xt[:], in_=lg_v[:, tch, n, :])
                    et = epool.tile([128, V], f32, tag="e")
                    nc.scalar.activation(
                        et[:], xt[:], AFT.Exp,
                        accum_out=se[:, k : k + 1],
                    )
                    # hierarchical x-domain max/argmax: 4 chunk-maxes, then
                    # top-8 of the slot, then the index of the max chunk
                    xv = xt[:].rearrange("p (c e) -> p c e", c=NCH)
                    nc.vector.reduce_max(
                        colmax[:, k * 8 : k * 8 + NCH], xv, axis=mybir.AxisListType.X
                    )
                    mxv = mx8[:, k * 8 : (k + 1) * 8]
                    cmv = colmax[:, k * 8 : (k + 1) * 8]
                    nc.vector.max(mxv, cmv)
                    nc.vector.max_index(c48[:, k * 8 : (k + 1) * 8], mxv, cmv)

                # per-group chunk ids, then a per-partition indirect gather of
                # each row's winning 512B chunk straight from DRAM
                nc.vector.scalar_tensor_tensor(
                    g32[:, k0 : k0 + G], c4s[:, k0 : k0 + G], 0,
                    base_flat[:, k0 : k0 + G], Alu.add, Alu.add,
                )
                for k in range(k0, k0 + G):
                    gt = gpool.tile([128, CH], f32, tag="g")
                    nc.gpsimd.indirect_dma_start(
                        gt[:],
                        None,
                        lg_flat,
                        bass.IndirectOffsetOnAxis(ap=g32[:, k : k + 1], axis=0),
                    )
                    nc.vector.max_index(
                        w8[:, k * 8 : (k + 1) * 8],
                        mx8[:, k * 8 : (k + 1) * 8],
                        gt[:],
                    )

            # ---- phase 1.5 (batched): maxlogp, argmax, staging, blocked
            # reload, masks, dedup, per-partition scan ----
            nc.scalar.activation(lnse[:], se[:], AFT.Ln)
            nc.vector.scalar_tensor_tensor(
                logp[:], mxs, 0.0, lnse[:], Alu.add, Alu.subtract
            )
            nc.vector.tensor_scalar(lm[:], iota_b[:], lens_sb[:, :], None, Alu.is_lt)
            for n in range(NLOC):
                sl = slice(n * NT, (n + 1) * NT)
                pr = slice(32 * n, 32 * n + 32)
                nc.vector.scalar_tensor_tensor(
                    amif[:, sl], c4s[:, sl], float(CH), ws[:, sl], Alu.mult, Alu.add
                )
                nc.sync.dma_start(out=st_ami[n : n + 1, :], in_=amif[:, sl])
                nc.sync.dma_start(out=st_logp[n : n + 1, :], in_=logp[:, sl])
                nc.sync.dma_start(out=ami_b[pr, :], in_=st_ami[n : n + 1, :])
                nc.vector.scalar_tensor_tensor(
                    nb[pr, :], ami_b[pr, :], float(BLANK), lm[pr, :],
                    Alu.not_equal, Alu.mult,
                )
                nc.vector.memset(prev0[32 * n : 32 * n + 1, :], -1.0)
                nc.sync.dma_start(
                    out=prev0[32 * n + 1 : 32 * n + 32, :],
                    in_=ami_b[32 * n : 32 * n + 31, 63:64],
                )
                nc.vector.scalar_tensor_tensor(
                    neq[pr, 1:], ami_b[pr, 1:], 0.0, ami_b[pr, :63],
                    Alu.add, Alu.not_equal,
                )
                nc.vector.scalar_tensor_tensor(
                    neq[pr, 0:1], ami_b[pr, 0:1], 0.0, prev0[pr, :],
                    Alu.add, Alu.not_equal,
                )
                nc.vector.scalar_tensor_tensor(
                    keep[pr, :], nb[pr, :], 0.0, neq[pr, :], Alu.add, Alu.mult
                )
                nc.vector.tensor_tensor_scan(
                    scb[pr, :], keep[pr, :], keep[pr, :], 0.0, Alu.add, Alu.bypass
                )
            nc.sync.dma_start(
                out=logp_b[:], in_=st_logp[:].rearrange("n (q j) -> (n q) j", j=64)
            )
            nc.vector.scalar_tensor_tensor(
                mtp[:], logp_b[:], 0.0, lm[:], Alu.add, Alu.mult,
                accum_out=mtpart[:],
            )
            # [128,1] -> [4,32] direct SBUF->SBUF (flatten orders align)
            sm_a = p2pool.tile([16, 32], f32, tag="sm_a", name="sm_a")
            nc.sync.dma_start(out=sm_a[0:NLOC, :], in_=scb[:, 63:64])
            sm_m = p2pool.tile([16, 32], f32, tag="sm_m", name="sm_m")
            nc.sync.dma_start(out=sm_m[0:NLOC, :], in_=mtpart[:])
            bts = p2pool.tile([16, 32], f32, tag="bts", name="bts")
            nc.vector.tensor_tensor_scan(
                bts[0:4, :], sm_a[0:4, :], sm_a[0:4, :], 0.0,
                Alu.add, Alu.bypass,
            )
            mts = p2pool.tile([16, 1], f32, tag="mts", name="mts")
            nc.vector.reduce_sum(
                mts[0:4, :], sm_m[0:4, :], axis=mybir.AxisListType.X
            )
            # exclusive prefix = inclusive - own; out_len = inclusive[31]
            exc = p2pool.tile([16, 32], f32, tag="exc", name="exc")
            nc.vector.scalar_tensor_tensor(
                exc[0:4, :], bts[0:4, :], 0.0, sm_a[0:4, :],
                Alu.add, Alu.subtract,
            )
            olf = p2pool.tile([16, 1], f32, tag="olf", name="olf")
            nc.vector.tensor_copy(olf[0:4, :], bts[0:4, 31:32])
            carry = p2pool.tile([128, 1], f32, tag="carry", name="carry")
            nc.sync.dma_start(out=carry[:], in_=exc[0:4, :])
            # safe_pos + 1 = keep * (scan + carry): 0 where dropped
            spp1 = tb("spp1")
            nc.vector.scalar_tensor_tensor(
                spp1[:], scb[:], carry[:, :], keep[:], Alu.add, Alu.mult
            )
            idx_hi_b = tb("idx_hi_b", i16)
            nc.vector.tensor_scalar(idx_hi_b[:], spp1[:], 1025.0, None, Alu.subtract)
            mhi = tb("mhi")
            nc.vector.tensor_scalar(mhi[:], spp1[:], 1025.0, None, Alu.is_ge)
            tlo = tb("tlo")
            nc.vector.scalar_tensor_tensor(
                tlo[:], mhi[:], -2048.0, spp1[:], Alu.mult, Alu.add
            )
            idx_lo_b = tb("idx_lo_b", i16)
            nc.vector.tensor_scalar(idx_lo_b[:], tlo[:], 1.0, None, Alu.subtract)
            # scatter argmax+1 so an untouched (zeroed) slot is identifiable
            dat16_b = tb("dat16_b", i16)
            nc.vector.tensor_scalar(dat16_b[:], ami_b[:], 1.0, None, Alu.add)

            # reshape to [16, T] rows for the per-partition local_scatter
            idx_hi = t4("idx_hi", i16)
            nc.gpsimd.memset(idx_hi[:, :], -1)
            idx_lo = t4("idx_lo", i16)
            nc.gpsimd.memset(idx_lo[:, :], -1)
            dat16 = t4("dat16", i16)
            nc.gpsimd.memset(dat16[:, :], 0)
            nc.sync.dma_start(out=idx_hi[0:NLOC, :], in_=idx_hi_b[:])
            nc.sync.dma_start(out=idx_lo[0:NLOC, :], in_=idx_lo_b[:])
            nc.sync.dma_start(out=dat16[0:NLOC, :], in_=dat16_b[:])

            cmp16 = t4("cmp16", i16)
            nc.gpsimd.local_scatter(
                cmp16[:, : T // 2], dat16[:], idx_lo[:],
                channels=16, num_elems=T // 2, num_idxs=T,
            )
            nc.gpsimd.local_scatter(
                cmp16[:, T // 2 :], dat16[:], idx_hi[:],
                channels=16, num_elems=T // 2, num_idxs=T,
            )

            if DEBUG:
                for nm, tile_ in (("dbg_c", cmp16), ("dbg_il", idx_lo), ("dbg_ih", idx_hi), ("dbg_dt", dat16)):
                    cnv = p2pool.tile([16, T], i32, tag="cnv_"+nm, name="cnv_"+nm)
                    nc.vector.tensor_copy(cnv[:], tile_[:])
                    nc.sync.dma_start(out={"dbg_c": dbg_c, "dbg_il": dbg_il, "dbg_ih": dbg_ih, "dbg_dt": dbg_dt}[nm].ap(), in_=cnv[:])
            cmp_b = tb("cmp_b", i16)
            nc.sync.dma_start(out=cmp_b[:], in_=cmp16[0:NLOC, :])
            cmpf = tb("cmpf")
            nc.vector.tensor_scalar(cmpf[:], cmp_b[:], 1.0, None, Alu.subtract)
            msel = tb("msel", i32)
            nc.vector.tensor_scalar(msel[:], cmpf[:], 0.0, None, Alu.is_ge)
            pf = tb("pf")
            nc.vector.select(pf[:], msel[:], cmpf[:], ami_b[:])
            pi = tb("pi", i32)
            nc.vector.tensor_copy(pi[:], pf[:])
            oli = p2pool.tile([16, 1], i32, tag="oli", name="oli")
            nc.vector.tensor_copy(oli[0:NLOC, :], olf[0:NLOC, :])

            nc.sync.dma_start(
                out=paths_o.ap().rearrange("n (q j) -> (n q) j", j=64), in_=pi[:]
            )
            nc.sync.dma_start(out=mt_o.ap(), in_=mts[0:NLOC, :])
            nc.sync.dma_start(out=ol_o.ap(), in_=oli[0:NLOC, :])

    return nc


def _get_nc():
    if "nc" not in _BUILT:
        nc = build_nc()
        nc.finalize()
        _BUILT["nc"] = nc
    return _BUILT["nc"]


_IOTA_B = (
    (np.arange(128)[:, None] % 32) * 64 + np.arange(64)[None, :]
).astype(np.float32)


def make_in_maps(logits, in_lens):
    logits = np.ascontiguousarray(np.asarray(logits, dtype=np.float32))
    lens = np.asarray(in_lens).astype(np.float32).reshape(N)
    in_maps = []
    for c in range(NCORES):
        sl = slice(NLOC * c, NLOC * (c + 1))
        in_maps.append(
            {
                "logits": np.ascontiguousarray(logits[:, sl, :]),
                "lens_f32": np.ascontiguousarray(lens[sl].reshape(NLOC, 1)),
                "iota_b": _IOTA_B,
                "lens_b": np.ascontiguousarray(
                    np.repeat(lens[sl], 32).reshape(128, 1)
                ),
            }
        )
    return in_maps


def kernel(logits, in_lens):
    from concourse.bass_utils import run_bass_kernel_spmd

    nc = _get_nc()
    in_maps = make_in_maps(logits, in_lens)
    res = run_bass_kernel_spmd(nc, in_maps, core_ids=list(range(NCORES))).results

    mt = np.concatenate([np.asarray(r["max_total"]).reshape(NLOC) for r in res])
    ol = np.concatenate([np.asarray(r["out_lens"]).reshape(NLOC) for r in res])
    paths = np.concatenate(
        [np.asarray(r["paths"]).reshape(NLOC, T) for r in res], axis=0
    )
    return (
        mt.astype(np.float32),
        np.ascontiguousarray(paths.T).astype(np.int32),
        ol.astype(np.int32),
    )
